# revision 9
# baseline (speedup 1.0000x reference)
"""DeepSeek-MoE block on 8 Trainium2 NeuronCores (Bass/Tile).

Sharding: expert-parallel. Each core owns 8 of the 64 routed experts; the 2
shared experts are computed per-core over that core's 128-token slice only.
Every core computes the full gate (softmax + top-6 threshold) for all 1024
tokens, then runs a masked-dense FFN over its experts: the per-(token, expert)
combine weight is zero for unselected experts, so no token dispatch is needed.
Core outputs are partial sums; the host unshard sums the 8 partials and adds
each core's shared-expert slice.

Precision: the gate matmul runs as a bf16 split-precision product
(Ghi@xhi + Ghi@xlo + Glo@xhi, accumulated in f32 PSUM) which matches fp32
scores to ~2^-16 relative, keeping the top-6 selection reference-exact; the
FFN (w1/w3/w2 matmuls, activations, combine weights, output partials) is
plain bf16 with f32 PSUM accumulation. Measured rel-err budget is 2e-2.

Fixed problem shapes (hardcoded per the harness contract):
  x [2, 512, 512] f32, g_w [64, 512], gate_bias [64],
  w1/w3 [66, 512, 64], w2 [66, 64, 512]; 2 shared + 64 routed, top-6.
"""

import sys

import numpy as np
import ml_dtypes

if "/opt/trn_rl_repo" not in sys.path:
    sys.path.insert(0, "/opt/trn_rl_repo")

import concourse.bass as bass
import concourse.mybir as mybir
import concourse.tile as tile
from concourse import bacc
from concourse.bass_utils import run_bass_kernel_spmd

DIM = 512
INTER = 64
N_SHARED = 2
N_ROUTED = 64
TOPK = 6
B, T = 2, 512
NTOK = B * T                 # 1024 tokens
N_CORES = 8
EXP_PER_CORE = N_ROUTED // N_CORES   # 8 routed experts per core
N_PAIR = EXP_PER_CORE // 2           # 4 routed expert pairs
N_TILE = NTOK // 128                 # 8 token tiles of 128
ST = 256                             # supertile token width for the FFN
N_ST = NTOK // ST                    # 4 supertiles
NCK = DIM // 128                     # 4 contraction chunks
HALF = NTOK // 2                     # gate processed in 2 token-halves

F32 = mybir.dt.float32
F32R = mybir.dt.float32r
BF16 = mybir.dt.bfloat16
AF = mybir.ActivationFunctionType
ALU = mybir.AluOpType

BF16_NP = ml_dtypes.bfloat16


def build_nc(uniform_bias=True):
    """Build the single-core Bass program (SPMD across 8 cores)."""
    nc = bacc.Bacc("TRN2", target_bir_lowering=False, debug=False)

    # ---- DRAM I/O (per-core values supplied by the host) ----
    # xb: [128, ck*1024] chunk-major per partition (host pre-layouts), bf16
    # xl: bf16 residual (x - bf16(x)) in the same layout, for the gate
    xb_d = nc.dram_tensor("xb", [128, NCK * NTOK], BF16, kind="ExternalInput")
    xl_d = nc.dram_tensor("xl", [128, NCK * NTOK], BF16, kind="ExternalInput")
    gwh_d = nc.dram_tensor("gwh", [128, NCK * N_ROUTED], BF16, kind="ExternalInput")
    gwl_d = nc.dram_tensor("gwl", [128, NCK * N_ROUTED], BF16, kind="ExternalInput")
    xs_d = nc.dram_tensor("xs", [128, NCK * 128], BF16, kind="ExternalInput")
    w1p_d = nc.dram_tensor("w1p", [128, NCK * N_PAIR * 128], BF16, kind="ExternalInput")
    w3p_d = nc.dram_tensor("w3p", [128, NCK * N_PAIR * 128], BF16, kind="ExternalInput")
    w2p_d = nc.dram_tensor("w2p", [128, N_PAIR * DIM], BF16, kind="ExternalInput")
    w1s_d = nc.dram_tensor("w1s", [128, NCK * 128], BF16, kind="ExternalInput")
    w3s_d = nc.dram_tensor("w3s", [128, NCK * 128], BF16, kind="ExternalInput")
    w2s_d = nc.dram_tensor("w2s", [128, DIM], BF16, kind="ExternalInput")
    esel_d = nc.dram_tensor("esel", [N_ROUTED, N_PAIR * 128], BF16, kind="ExternalInput")
    identr_d = nc.dram_tensor("identr", [128, 128], F32R, kind="ExternalInput")
    identb_d = nc.dram_tensor("identb", [128, 128], BF16, kind="ExternalInput")
    if not uniform_bias:
        biasb_d = nc.dram_tensor("biasb", [128, N_TILE * N_ROUTED], F32, kind="ExternalInput")
    pout_d = nc.dram_tensor("pout", [NTOK, DIM], BF16, kind="ExternalOutput")
    posh_d = nc.dram_tensor("posh", [128, DIM], BF16, kind="ExternalOutput")

    with tile.TileContext(nc) as tc:
        with (
            tc.tile_pool(name="const", bufs=1) as cpool,
            tc.tile_pool(name="gate", bufs=1) as gpool,
            tc.tile_pool(name="act", bufs=4) as apool,
            tc.tile_pool(name="psA", bufs=2, space="PSUM") as psA,
            tc.tile_pool(name="psO", bufs=1, space="PSUM") as psO,
        ):
            # ---- PE warmup: ~4.5us of dummy matmuls promotes the HAM
            # throttle to K=8/8 while the DMA loads stream in ----
            warm_sb = cpool.tile([128, 128], BF16, tag="warm")
            nc.vector.memset(warm_sb[:], 1.0)
            warm_mv = cpool.tile([128, 512], BF16, tag="warmv")
            nc.vector.memset(warm_mv[:], 1.0)
            warm_ps = psA.tile([128, 512], F32, tag="wb", name="warm_ps")
            for _ in range(12):
                nc.tensor.matmul(warm_ps[:], warm_sb[:], warm_mv[:], start=True, stop=True)

            # ---- persistent SBUF loads, ordered by first use ----
            # sync queue: gate weights -> xb h0 -> xl h0 -> w2p -> xb h1
            gwh_sb = cpool.tile([128, NCK * N_ROUTED], BF16, tag="gwh")
            nc.sync.dma_start(gwh_sb[:], gwh_d.ap())
            gwl_sb = cpool.tile([128, NCK * N_ROUTED], BF16, tag="gwl")
            nc.sync.dma_start(gwl_sb[:], gwl_d.ap())
            xb_sb = cpool.tile([128, NCK * NTOK], BF16, tag="xb")
            xl_sb = cpool.tile([128, NCK * NTOK], BF16, tag="xl")
            for ck in range(NCK):
                lo = ck * NTOK
                nc.sync.dma_start(xb_sb[:, lo : lo + HALF], xb_d.ap()[:, lo : lo + HALF])
            for ck in range(NCK):
                lo = ck * NTOK
                nc.sync.dma_start(xl_sb[:, lo : lo + HALF], xl_d.ap()[:, lo : lo + HALF])
            # scalar queue: FFN weights -> xl h1 -> w2s
            w1p_sb = cpool.tile([128, NCK * N_PAIR * 128], BF16, tag="w1p")
            nc.scalar.dma_start(w1p_sb[:], w1p_d.ap())
            w3p_sb = cpool.tile([128, NCK * N_PAIR * 128], BF16, tag="w3p")
            nc.scalar.dma_start(w3p_sb[:], w3p_d.ap())
            for ck in range(NCK):
                lo = ck * NTOK + HALF
                nc.scalar.dma_start(xb_sb[:, lo : lo + HALF], xb_d.ap()[:, lo : lo + HALF])
            w2p_sb = cpool.tile([128, N_PAIR * DIM], BF16, tag="w2p")
            nc.sync.dma_start(w2p_sb[:], w2p_d.ap())
            for ck in range(NCK):
                lo = ck * NTOK + HALF
                nc.scalar.dma_start(xl_sb[:, lo : lo + HALF], xl_d.ap()[:, lo : lo + HALF])
            w2s_sb = cpool.tile([128, DIM], BF16, tag="w2s")
            nc.scalar.dma_start(w2s_sb[:], w2s_d.ap())
            # gpsimd queue: small constants + shared-expert inputs
            identr_sb = cpool.tile([128, 128], F32R, tag="identr")
            nc.gpsimd.dma_start(identr_sb[:], identr_d.ap())
            identb_sb = cpool.tile([128, 128], BF16, tag="identb")
            nc.gpsimd.dma_start(identb_sb[:], identb_d.ap())
            esel_sb = cpool.tile([N_ROUTED, N_PAIR * 128], BF16, tag="esel")
            nc.gpsimd.dma_start(esel_sb[:], esel_d.ap())
            xs_sb = cpool.tile([128, NCK * 128], BF16, tag="xs")
            nc.gpsimd.dma_start(xs_sb[:], xs_d.ap())
            w1s_sb = cpool.tile([128, NCK * 128], BF16, tag="w1s")
            nc.gpsimd.dma_start(w1s_sb[:], w1s_d.ap())
            w3s_sb = cpool.tile([128, NCK * 128], BF16, tag="w3s")
            nc.gpsimd.dma_start(w3s_sb[:], w3s_d.ap())
            if not uniform_bias:
                biasb_sb = cpool.tile([128, N_TILE * N_ROUTED], F32, tag="biasb")
                nc.gpsimd.dma_start(biasb_sb[:], biasb_d.ap())

            wt_sb = gpool.tile([N_ROUTED, NTOK], BF16, tag="wt")

            g = lambda tag, w=N_TILE: gpool.tile([128, w * N_ROUTED], F32, tag=tag, name=tag)
            sm = lambda tag, w=N_TILE: gpool.tile([128, w], F32, tag=tag, name=tag)

            def r3(t):
                return t.rearrange("p (t e) -> p t e", e=N_ROUTED)

            def bc8(t):
                # [128, 8] scalar-per-tile -> broadcast [128, 8, 64]
                return t.unsqueeze(-1).to_broadcast([128, N_TILE, N_ROUTED])

            # ======== gate head: split-precision scores^T per half ============
            scT = gpool.tile([N_ROUTED, NTOK], F32R, tag="scT")
            exps = g("exps")
            rsum = sm("rsum")

            def gate_scores(h):
                base = h * HALF
                ps = psA.tile([N_ROUTED, HALF], F32, tag="h1", name=f"scTps{h}")
                passes = [(gwh_sb, xb_sb), (gwl_sb, xb_sb), (gwh_sb, xl_sb)]
                n = len(passes) * NCK
                i = 0
                for gw, xx in passes:
                    for ck in range(NCK):
                        nc.tensor.matmul(
                            ps[:],
                            gw[:, ck * N_ROUTED : (ck + 1) * N_ROUTED],
                            xx[:, ck * NTOK + base : ck * NTOK + base + HALF],
                            start=(i == 0),
                            stop=(i == n - 1),
                        )
                        i += 1
                nc.vector.tensor_copy(scT[:, base : base + HALF], ps[:])

            def gate_tp(h):
                # transpose 128-token tiles (f32r: 1.5 cyc/row), exp from PSUM
                for tt in range(4 * h, 4 * h + 4):
                    tps = psA.tile([128, N_ROUTED], F32R, tag="h3", name=f"tps{tt}")
                    nc.tensor.transpose(
                        tps[:],
                        scT[:, tt * 128 : (tt + 1) * 128],
                        identr_sb[0:N_ROUTED, 0:N_ROUTED],
                    )
                    # |scores| <= ~2.5 here, so exp needs no max-subtraction
                    nc.scalar.activation(
                        exps[:, tt * N_ROUTED : (tt + 1) * N_ROUTED],
                        tps[:].bitcast(F32),
                        AF.Exp,
                        accum_out=rsum[:, tt : tt + 1],
                    )

            # ======== gate chain: Max8 threshold, batched combine ============
            def gate_chain():
                rinv = sm("rinv")
                nc.vector.reciprocal(rinv[:], rsum[:])
                if uniform_bias:
                    sel = exps
                else:
                    probs0 = g("probs0")
                    nc.vector.tensor_tensor(r3(probs0[:]), r3(exps[:]), bc8(rinv[:]), op=ALU.mult)
                    sel = g("biased")
                    nc.vector.tensor_tensor(sel[:], probs0[:], biasb_sb[:], op=ALU.add)
                m8a = gpool.tile([128, N_TILE * 8], F32, tag="m8a", name="m8a")
                for tt in range(N_TILE):
                    nc.vector.max(m8a[:, tt * 8 : (tt + 1) * 8], sel[:, tt * N_ROUTED : (tt + 1) * N_ROUTED])
                # threshold scalar (6th max) per tile, broadcast over experts
                m8b = (
                    m8a.rearrange("p (t k) -> p t k", k=8)[:, :, 5:6]
                    .to_broadcast([128, N_TILE, N_ROUTED])
                )
                msk = g("msk")
                nc.vector.tensor_tensor(r3(msk[:]), r3(sel[:]), m8b, op=ALU.is_ge)
                wcomb = gpool.tile([128, N_TILE * N_ROUTED], BF16, tag="wcomb", name="wcomb")
                if uniform_bias:
                    mskr = g("mskr")
                    nc.vector.tensor_tensor(r3(mskr[:]), r3(msk[:]), bc8(rinv[:]), op=ALU.mult)
                    nc.vector.tensor_tensor(wcomb[:], exps[:], mskr[:], op=ALU.mult)
                else:
                    nc.vector.tensor_tensor(wcomb[:], probs0[:], msk[:], op=ALU.mult)

                for tt in range(N_TILE):
                    wtp = psA.tile([N_ROUTED, 128], BF16, tag="h3", name=f"wtp{tt}")
                    nc.tensor.transpose(
                        wtp[:], wcomb[:, tt * N_ROUTED : (tt + 1) * N_ROUTED], identb_sb[:]
                    )
                    nc.vector.tensor_copy(
                        wt_sb[:, tt * 128 : (tt + 1) * 128], wtp[:]
                    )

            # ======== FFN fronts (gate-independent): h13 -> silu -> prod =====
            prods = {}

            def ffn_front(q):
                t0 = q * 2 * ST
                for p in range(N_PAIR):
                    h1 = psA.tile([128, 2 * ST], F32, tag="h1")
                    h3 = psA.tile([128, 2 * ST], F32, tag="h3")
                    for ck in range(NCK):
                        xck = xb_sb[:, ck * NTOK + t0 : ck * NTOK + t0 + 2 * ST]
                        nc.tensor.matmul(
                            h1[:],
                            w1p_sb[:, (ck * N_PAIR + p) * 128 : (ck * N_PAIR + p + 1) * 128],
                            xck,
                            start=(ck == 0),
                            stop=(ck == NCK - 1),
                        )
                        nc.tensor.matmul(
                            h3[:],
                            w3p_sb[:, (ck * N_PAIR + p) * 128 : (ck * N_PAIR + p + 1) * 128],
                            xck,
                            start=(ck == 0),
                            stop=(ck == NCK - 1),
                        )
                    silu = apool.tile([128, 2 * ST], BF16, tag="silu", bufs=10, name=f"silu{q}_{p}")
                    nc.scalar.activation(silu[:], h1[:], AF.Silu)
                    # gate-independent product; h3 read straight from PSUM
                    aT1 = apool.tile([128, 2 * ST], BF16, tag="aT1", bufs=10, name=f"aT1{q}_{p}")
                    nc.vector.tensor_tensor(aT1[:], silu[:], h3[:], op=ALU.mult)
                    prods[(q, p)] = aT1

            def ffn_front_shared():
                h1 = psA.tile([128, 2 * ST], F32, tag="h1", name="h1sh")
                h3 = psA.tile([128, 2 * ST], F32, tag="h3", name="h3sh")
                for ck in range(NCK):
                    xck = xs_sb[:, ck * 128 : (ck + 1) * 128]
                    nc.tensor.matmul(
                        h1[:, 0:128],
                        w1s_sb[:, ck * 128 : (ck + 1) * 128],
                        xck,
                        start=(ck == 0),
                        stop=(ck == NCK - 1),
                    )
                    nc.tensor.matmul(
                        h3[:, 0:128],
                        w3s_sb[:, ck * 128 : (ck + 1) * 128],
                        xck,
                        start=(ck == 0),
                        stop=(ck == NCK - 1),
                    )
                silu = apool.tile([128, 128], BF16, tag="silu", bufs=10, name="silush")
                nc.scalar.activation(silu[:], h1[:, 0:128], AF.Silu)
                aT1 = apool.tile([128, 128], BF16, tag="aT1", bufs=10, name="aT1sh")
                nc.vector.tensor_tensor(aT1[:], silu[:], h3[:, 0:128], op=ALU.mult)
                return aT1

            # ======== FFN backs (gate-dependent): wb -> aT -> combine -> out ===
            aTs = {}

            def ffn_back_head(q):
                t0 = q * 2 * ST
                for p in range(N_PAIR):
                    wb = psA.tile([128, 2 * ST], F32, tag="wb")
                    nc.tensor.matmul(
                        wb[:],
                        esel_sb[:, p * 128 : (p + 1) * 128],
                        wt_sb[:, t0 : t0 + 2 * ST],
                        start=True,
                        stop=True,
                    )
                    aT1 = prods[(q, p)]
                    aT = apool.tile([128, 2 * ST], BF16, tag="aT", bufs=6, name=f"aT{q}_{p}")
                    nc.vector.tensor_tensor(aT[:], aT1[:], wb[:], op=ALU.mult)
                    aTs[(q, p)] = aT

            def ffn_back(st):
                t0 = st * ST
                outp = [
                    psO.tile([128, DIM], F32, name=f"outp{st}_{s}", tag=f"out{s}")
                    for s in range(ST // 128)
                ]
                for p in range(N_PAIR):
                    aT = aTs[(st // 2, p)]
                    off = (st % 2) * ST
                    for s in range(ST // 128):
                        nc.tensor.matmul(
                            outp[s][:],
                            aT[:, off + s * 128 : off + (s + 1) * 128],
                            w2p_sb[:, p * DIM : (p + 1) * DIM],
                            start=(p == 0),
                            stop=(p == N_PAIR - 1),
                        )
                for s in range(ST // 128):
                    osb = apool.tile([128, DIM], BF16, tag="osb", name=f"osb{st}_{s}")
                    if s == 0:
                        nc.scalar.copy(osb[:], outp[s][:])
                    else:
                        nc.vector.tensor_copy(osb[:], outp[s][:])
                    eng = nc.sync if s == 0 else nc.scalar
                    eng.dma_start(
                        pout_d.ap()[t0 + s * 128 : t0 + (s + 1) * 128, :], osb[:]
                    )

            def ffn_back_shared(aT1_sh):
                outp = psO.tile([128, DIM], F32, name="outpsh", tag="out0")
                nc.tensor.matmul(outp[:], aT1_sh[:], w2s_sb[:], start=True, stop=True)
                osh = apool.tile([128, DIM], BF16, tag="osb", name="osh")
                nc.scalar.copy(osh[:], outp[:])
                nc.scalar.dma_start(posh_d.ap()[:, :], osh[:])

            # ---- schedule: gate h0 early, front(0) fills the xl-h1 DMA window
            gate_scores(0)
            gate_tp(0)
            ffn_front(0)
            aT1_sh = ffn_front_shared()
            gate_scores(1)
            gate_tp(1)
            gate_chain()
            ffn_back_head(0)
            ffn_back(0)
            ffn_back(1)
            ffn_front(1)
            ffn_back_head(1)
            ffn_back(2)
            ffn_back(3)
            ffn_back_shared(aT1_sh)

    nc.compile()
    return nc


def make_core_inputs(x, g_w, gate_bias, w1, w2, w3, uniform=None):
    """Host-side sharding/layout prep. Returns list of 8 per-core input maps."""
    x = np.ascontiguousarray(np.asarray(x, dtype=np.float32)).reshape(NTOK, DIM)
    g_w = np.asarray(g_w, dtype=np.float32)
    gate_bias = np.asarray(gate_bias, dtype=np.float32)
    w1 = np.asarray(w1, dtype=np.float32)
    w2 = np.asarray(w2, dtype=np.float32)
    w3 = np.asarray(w3, dtype=np.float32)
    if uniform is None:
        uniform = bool(np.ptp(gate_bias) == 0.0)

    # xt host layout: [128 partitions, ck*1024] with xt[p, ck*1024+t] = x[t, ck*128+p]
    xt = np.ascontiguousarray(
        x.T.reshape(NCK, 128, NTOK).transpose(1, 0, 2).reshape(128, NCK * NTOK)
    )
    xb = xt.astype(BF16_NP)
    xl = (xt - xb.astype(np.float32)).astype(BF16_NP)   # bf16 residual of x
    bias_shift = gate_bias - gate_bias.min() + 1.0      # keep biased scores > 0
    identr = np.eye(128, dtype=np.float32)
    identb = np.eye(128, dtype=BF16_NP)
    # esel[k, p*128 + j] selects wt row k into broadcast partitions j of pair p
    esel = np.zeros((N_ROUTED, N_PAIR * 128), dtype=BF16_NP)
    for p in range(N_PAIR):
        esel[2 * p, p * 128 : p * 128 + 64] = 1.0
        esel[2 * p + 1, p * 128 + 64 : (p + 1) * 128] = 1.0

    # shared experts (global slots 0, 1) concatenated along the inter axis
    w1sh = np.concatenate([w1[0], w1[1]], axis=1)        # [512, 128]
    w3sh = np.concatenate([w3[0], w3[1]], axis=1)
    w2sh = np.concatenate([w2[0], w2[1]], axis=0)        # [128, 512]
    w1s = np.ascontiguousarray(
        w1sh.reshape(NCK, 128, 128).transpose(1, 0, 2).reshape(128, -1)
    ).astype(BF16_NP)
    w3s = np.ascontiguousarray(
        w3sh.reshape(NCK, 128, 128).transpose(1, 0, 2).reshape(128, -1)
    ).astype(BF16_NP)
    w2s = np.ascontiguousarray(w2sh).astype(BF16_NP)

    in_maps = []
    for c in range(N_CORES):
        mine = list(range(EXP_PER_CORE * c, EXP_PER_CORE * (c + 1)))
        perm = mine + [e for e in range(N_ROUTED) if e not in mine]
        # gwt host layout [128, ck*64]: gwt[p, ck*64+e] = g_w[perm[e], ck*128+p]
        gwt_c = np.ascontiguousarray(
            g_w[perm].T.reshape(NCK, 128, N_ROUTED).transpose(1, 0, 2).reshape(128, -1)
        )
        gwh = gwt_c.astype(BF16_NP)
        gwl = (gwt_c - gwh.astype(np.float32)).astype(BF16_NP)

        # this core's 128-token slice of xb, chunk-major (for the shared experts)
        xs_c = np.ascontiguousarray(
            np.concatenate(
                [xb[:, ck * NTOK + 128 * c : ck * NTOK + 128 * (c + 1)] for ck in range(NCK)],
                axis=1,
            )
        )

        # routed expert slots (global idx 2+e), paired along the inter axis
        slots = [2 + e for e in mine]
        w1r = w1[slots]                                  # [8, 512, 64]
        w3r = w3[slots]
        w2r = w2[slots]                                  # [8, 64, 512]
        w1pair = np.stack(
            [np.concatenate([w1r[2 * p], w1r[2 * p + 1]], axis=1) for p in range(N_PAIR)]
        )  # [4, 512, 128]
        w3pair = np.stack(
            [np.concatenate([w3r[2 * p], w3r[2 * p + 1]], axis=1) for p in range(N_PAIR)]
        )
        w2pair = np.stack(
            [np.concatenate([w2r[2 * p], w2r[2 * p + 1]], axis=0) for p in range(N_PAIR)]
        )  # [4, 128, 512]

        # SBUF layouts: w1p [128p, ck, pair, 128], w2p [128p, pair*512]
        w1p = np.ascontiguousarray(
            w1pair.reshape(N_PAIR, NCK, 128, 128).transpose(2, 1, 0, 3).reshape(128, -1)
        ).astype(BF16_NP)
        w3p = np.ascontiguousarray(
            w3pair.reshape(N_PAIR, NCK, 128, 128).transpose(2, 1, 0, 3).reshape(128, -1)
        ).astype(BF16_NP)
        w2p = np.ascontiguousarray(w2pair.transpose(1, 0, 2).reshape(128, -1)).astype(BF16_NP)

        m = {
            "xb": xb,
            "xl": xl,
            "gwh": gwh,
            "gwl": gwl,
            "xs": xs_c,
            "w1p": w1p,
            "w3p": w3p,
            "w2p": w2p,
            "w1s": w1s,
            "w3s": w3s,
            "w2s": w2s,
            "esel": esel,
            "identr": identr,
            "identb": identb,
        }
        if not uniform:
            m["biasb"] = np.tile(bias_shift, (128, N_TILE))  # [128, 512]
        in_maps.append(m)
    return in_maps


_NC_CACHE = {}


def kernel(x, g_w, gate_bias, w1, w2, w3):
    uniform = bool(np.ptp(np.asarray(gate_bias, dtype=np.float32)) == 0.0)
    if uniform not in _NC_CACHE:
        _NC_CACHE[uniform] = build_nc(uniform_bias=uniform)
    nc = _NC_CACHE[uniform]
    in_maps = make_core_inputs(x, g_w, gate_bias, w1, w2, w3, uniform=uniform)
    res = run_bass_kernel_spmd(nc, in_maps, list(range(N_CORES)))
    out = np.zeros((NTOK, DIM), dtype=np.float32)
    for c, r in enumerate(res.results):
        out += np.asarray(r["pout"], dtype=np.float32)
        out[128 * c : 128 * (c + 1)] += np.asarray(r["posh"], dtype=np.float32)
    return out.reshape(B, T, DIM)
